# revision 21
# baseline (speedup 1.0000x reference)
"""Causal self-attention (B=2, T=2048, D=1024, H=16) on 8 TRN2 NeuronCores.

Sharding: data-parallel over batch (2) x tensor-parallel over head groups (4).
Each core handles 1 batch x 4 heads: Wq/Wk/Wv column-sharded, Wo row-sharded;
each core emits a partial (T, D) output and the host sums 4 partials per batch.

v2 design (vs the fp32r baseline):
  - x is transposed on the HOST and shipped as bf16 [P, DC, T] partition-major,
    eliminating all 128 PE transposes and their PSUM evictions.
  - All matmul operands are bf16 (fp32 PSUM accumulation): enables the PE's
    fast-weight-load path (fp32r blocks FWL), halves eviction/DMA bytes.
  - No mask-inject matmuls: diagonal S tiles compute only columns [d, 512);
    exp is restricted to the written PSUM region (split calls on diagonal
    k-pairs), and the in-tile causal triangle is applied post-exp as a bf16
    tensor_mul with a host-provided [128,128] upper-triangular mask.
  - All PSUM evictions run on DVE; the scalar engine does (almost) only exp.
  - Softmax denominators ride in V' ones-columns (parity layout: even heads
    col 64 / rows 0-63 data, odd heads col 0 / rows 64-127 data); per (qs,h)
    the denom row is folded via SBUF DMA, reciprocal'd on DVE, unfolded and
    rank-1-broadcast on PE, then fused into the oT eviction multiply.
"""

import sys, os, types

sys.path.insert(0, "/opt/trn_rl_repo")

import numpy as np
from contextlib import ExitStack

import concourse.bass as bass
import concourse.mybir as mybir
import concourse.tile as tile
from concourse import bacc

B, T, D, H = 2, 2048, 1024, 16
DH = D // H          # 64
NCORES = 8
HG = 4               # heads per core
F = HG * DH          # 256 local features per core
P = 128
F32 = mybir.dt.float32
F32R = mybir.dt.float32r
BF16 = mybir.dt.bfloat16
FP8 = mybir.dt.float8e4

TT = T // P          # 16 t-tiles
QS = T // 512        # 4 q-slabs
DC = D // P          # 8 d-chunks

LAST_RESULTS = None  # BassKernelResults of the most recent hardware run


def _install_ntff_hook():
    if "antenv.axon_hooks" in sys.modules:
        return
    try:
        import antenv
        from trn_agent_boot.trn_boot import _ntff_profile_via_ctypes

        m = types.ModuleType("antenv.axon_hooks")
        h = _ntff_profile_via_ctypes("/opt/axon/libaxon_pjrt.so")
        m.get_axon_ntff_profile_hook = lambda: h
        m.set_axon_ntff_profile_hook = lambda hh: None
        sys.modules["antenv.axon_hooks"] = m
        antenv.axon_hooks = m
    except Exception:
        pass


def build_nc():
    nc = bacc.Bacc("TRN2", target_bir_lowering=False, debug=False)

    xt_d = nc.dram_tensor("xt", [P, DC, T], BF16, kind="ExternalInput").ap()
    wq_d = nc.dram_tensor("wq", [P, DC, F], BF16, kind="ExternalInput").ap()
    wk_d = nc.dram_tensor("wk", [P, DC, F], BF16, kind="ExternalInput").ap()
    wv_d = nc.dram_tensor("wv", [P, DC, F], BF16, kind="ExternalInput").ap()
    wo_d = nc.dram_tensor("wo", [P, 2, D], BF16, kind="ExternalInput").ap()
    tri_d = nc.dram_tensor("tri", [P, P], BF16, kind="ExternalInput").ap()
    vpi_d = nc.dram_tensor("vpi", [P, TT, HG, P], BF16, kind="ExternalInput").ap()
    y_d = nc.dram_tensor("y", [T, D], BF16, kind="ExternalOutput").ap()

    with tile.TileContext(nc) as tc, ExitStack() as ctx:
        const = ctx.enter_context(tc.tile_pool(name="const", bufs=1))
        wpool = ctx.enter_context(tc.tile_pool(name="wpool", bufs=1))
        qkv = ctx.enter_context(tc.tile_pool(name="qkv", bufs=1))
        xsl = ctx.enter_context(tc.tile_pool(name="xsl", bufs=2))
        sp_ps = ctx.enter_context(tc.tile_pool(name="sp_ps", bufs=2, space="PSUM"))
        prj = ctx.enter_context(tc.tile_pool(name="prj", bufs=2, space="PSUM"))
        o_ps = ctx.enter_context(tc.tile_pool(name="o_ps", bufs=2, space="PSUM"))
        ptp = ctx.enter_context(tc.tile_pool(name="ptp", bufs=4))
        stg = ctx.enter_context(tc.tile_pool(name="stg", bufs=6))
        ysb = ctx.enter_context(tc.tile_pool(name="ysb", bufs=4))

        # ---- constants / warmups ----
        tri = const.tile([P, P], BF16, name="tri")
        # dummy matmul burst: ~4.5us of PE activity during the DMA-bound
        # preamble flips the HAM clock gate to 8/8 before real work arrives
        wsrc = const.tile([P, P], BF16, name="wsrc")
        nc.vector.memset(wsrc[:], 0.0)
        wps = prj.tile([P, 512], F32, name="wps", tag="prj")
        for _ in range(40):
            nc.tensor.matmul(out=wps[:, 0:P], lhsT=wsrc[:], rhs=wsrc[:],
                             start=True, stop=True)
        # touch Exp early so the ACT table load happens in the idle preamble
        warm_src = const.tile([1, 1], F32, name="warm_src")
        nc.vector.memset(warm_src[:], 0.0)
        warm = const.tile([1, 1], F32, name="warm")
        nc.scalar.activation(warm[:], warm_src[:], mybir.ActivationFunctionType.Exp)
        # touch partition_broadcast early so the gpsimd library IRAM load
        # (~6us) happens in the idle preamble
        wpb = const.tile([P, 64], BF16, name="wpb")
        nc.vector.memset(wpb[0:1, :], 1.0)
        nc.gpsimd.partition_broadcast(wpb[:, :], wpb[0:1, :])
        # denominator-broadcast selectors for the K=1 PE path (base-64 rows):
        # bcast = oselB[r]^T (x) inv[r]; even heads r=64 -> out rows 0:64
        oselB = const.tile([65, 2, P], BF16, name="oselB")
        nc.vector.memset(oselB[0:1, :, :], 0.0)
        nc.vector.memset(oselB[64:65, :, :], 0.0)
        nc.vector.memset(oselB[64:65, 0, 0:64], 1.0)

        # ---- persistent tensors ----
        wq_s = wpool.tile([P, DC, F], BF16, name="wq_s")
        wk_s = wpool.tile([P, DC, F], BF16, name="wk_s")
        wv_s = wpool.tile([P, DC, F], BF16, name="wv_s")
        wo2 = wpool.tile([P, 2, D], BF16, name="wo2")
        qT = qkv.tile([P, 2, T], BF16, name="qT")        # [2 heads x dh, jb, t]
        kTz0 = qkv.tile([P, 2, T], BF16, name="kTz0")    # [k_even; 0]
        kTz1 = qkv.tile([P, 2, T], BF16, name="kTz1")    # [0; k_odd]
        vp = qkv.tile([P, TT, HG, P], BF16, name="vp")   # padded V', parity layouts
        oT = qkv.tile([P, 2, T], BF16, name="oT")        # normalized o^T [f, t]

        # ---- preamble DMAs + fills ----
        # Only what gates the first projections ships immediately: x slab 0
        # (sync ring), wq/wk/tri (scalar ring), vp init for q-slab 0's
        # k-tiles (gpsimd ring).  Everything else (x slab 1, wv, wo, rest of
        # vp init) is sequenced behind slab 0's arrival with tiny
        # dependency-creating writes, so the SDMA engines don't split early
        # bandwidth 6 ways and delay the pipeline start.
        xs_next = xsl.tile([P, DC, 512], BF16, name="xs")
        for c2 in range(4):
            nc.sync.dma_start(out=xs_next[:, 2 * c2:2 * c2 + 2, :],
                              in_=xt_d[:, 2 * c2:2 * c2 + 2, 0:512])
        nc.scalar.dma_start(out=wq_s[:, 0:4], in_=wq_d[:, 0:4])
        nc.scalar.dma_start(out=wq_s[:, 4:8], in_=wq_d[:, 4:8])
        nc.scalar.dma_start(out=wk_s[:, 0:4], in_=wk_d[:, 0:4])
        nc.scalar.dma_start(out=wk_s[:, 4:8], in_=wk_d[:, 4:8])
        nc.scalar.dma_start(out=tri[:], in_=tri_d[:])
        nc.vector.memset(kTz0[64:128, :, :], 0.0)
        nc.vector.memset(kTz1[0:64, :, :], 0.0)
        xs0 = xs_next
        for tgt in (vp[0:1, 0, 0, 0:1], vp[0:1, 4, 0, 0:1],
                    wv_s[0:1, 0, 0:1], wo2[0:1, 0, 0:1]):
            nc.vector.tensor_copy(tgt, xs0[0:1, 0, 0:1])
        nc.gpsimd.dma_start(out=vp[:, 0:4], in_=vpi_d[:, 0:4])
        nc.gpsimd.dma_start(out=vp[:, 4:TT], in_=vpi_d[:, 4:TT])
        nc.scalar.dma_start(out=wv_s[:], in_=wv_d[:])
        nc.scalar.dma_start(out=wo2[:], in_=wo_d[:])

        # ---- emission helpers ----
        def emit_slab(ts):
            nonlocal xs_next
            xs = xs_next
            if ts + 1 < QS:
                xs_next = xsl.tile([P, DC, 512], BF16, name="xs")
                if ts == 0:
                    # keep slab 1's transfer out of the startup DMA window
                    nc.vector.tensor_copy(xs_next[0:1, 0, 0:1], xs[0:1, 0, 0:1])
                nc.sync.dma_start(out=xs_next[:], in_=xt_d[:, :, (ts + 1) * 512:(ts + 2) * 512])
            sl = slice(ts * 512, (ts + 1) * 512)

            def emit_proj(w_s, which, jb):
                pp = prj.tile([P, 512], F32, name="pp", tag="prj")
                for dc in range(DC):
                    nc.tensor.matmul(
                        out=pp[:],
                        lhsT=w_s[:, dc, jb * P:(jb + 1) * P],
                        rhs=xs[:, dc, :],
                        start=(dc == 0),
                        stop=(dc == DC - 1),
                    )
                if which == "q":
                    nc.vector.tensor_copy(qT[:, jb, sl], pp[:])
                else:
                    nc.vector.tensor_copy(kTz0[0:64, jb, sl], pp[0:64, :])
                    nc.vector.tensor_copy(kTz1[64:128, jb, sl], pp[64:128, :])

            for jb in range(2):
                emit_proj(wq_s, "q", jb)
            deferred = []
            if ts == 0:
                for jb in range(2):
                    emit_proj(wk_s, "k", jb)
            else:
                for jb in range(2):
                    deferred.append(lambda b=jb: emit_proj(wk_s, "k", b))

            def emit_v(j, tt):
                pv = prj.tile([P, F], F32, name="pv", tag="prj")
                for dc in range(DC):
                    nc.tensor.matmul(
                        out=pv[:],
                        lhsT=xs[:, dc, j * P:(j + 1) * P],
                        rhs=wv_s[:, dc, :],
                        start=(dc == 0),
                        stop=(dc == DC - 1),
                    )
                pvv = pv[:].rearrange("p (hp par dh) -> p hp par dh", hp=2, par=2, dh=DH)
                ve = vp[:, tt, :, :].rearrange("p (hp par) c -> p hp par c", par=2)
                nc.vector.tensor_copy(ve[:, :, 0, 0:DH], pvv[:, :, 0, :])
                nc.vector.tensor_copy(ve[:, :, 1, DH:P], pvv[:, :, 1, :])
            return deferred + [(lambda a=j_, b=tt_: emit_v(a, b)) for j_, tt_ in enumerate(range(4 * ts, 4 * ts + 4))]

        state = {}

        def emit_S(qs, h, kp):
            jbh, par = h // 2, h % 2
            kTz = kTz0 if par == 0 else kTz1
            q0 = qs * 512
            spair = sp_ps.tile([P, 1024], F32, name="spair", tag="sp")
            for half in range(2):
                kt = 2 * kp + half
                k0 = kt * P
                sreg = spair[:, half * 512:(half + 1) * 512]
                lhsk = kTz[:, jbh, k0:k0 + P]
                rhsq = qT[:, jbh, :]
                d = k0 - q0
                if d > 0:
                    nc.tensor.matmul(out=sreg[:, d:512], lhsT=lhsk,
                                     rhs=rhsq[:, q0 + d:q0 + 512],
                                     start=True, stop=True)
                else:
                    nc.tensor.matmul(out=sreg, lhsT=lhsk,
                                     rhs=rhsq[:, q0:q0 + 512],
                                     start=True, stop=True)
            state[(qs, h, kp)] = spair

        def emit_exp(qs, h, kp):
            spair = state[(qs, h, kp)]
            q0 = qs * 512
            pt = ptp.tile([P, 1024], BF16, name="pt")
            d1 = (2 * kp + 1) * P - q0
            if d1 > 0:
                # diagonal pair: exp only the written PSUM regions
                d0 = max(d1 - P, 0)
                nc.scalar.activation(pt[:, d0:512], spair[:, d0:512],
                                     mybir.ActivationFunctionType.Exp, scale=0.125)
                nc.scalar.activation(pt[:, 512 + d1:1024], spair[:, 512 + d1:1024],
                                     mybir.ActivationFunctionType.Exp, scale=0.125)
            else:
                nc.scalar.activation(pt[:], spair[:],
                                     mybir.ActivationFunctionType.Exp, scale=0.125)
            # in-tile causal triangle on diagonal k-tiles
            for half in range(2):
                d = (2 * kp + half) * P - q0
                if d >= 0:
                    c = half * 512 + d
                    nc.vector.tensor_mul(pt[:, c:c + P], pt[:, c:c + P], tri[:])
            state[(qs, h, kp)] = (spair, pt)

        def emit_AV(qs, h, kp, nkt):
            _, pt = state.pop((qs, h, kp))
            q0 = qs * 512
            if kp == 0:
                state[(qs, h)] = o_ps.tile([P, 512], F32, name="opsum")
            opsum = state[(qs, h)]
            for half in range(2):
                kt = 2 * kp + half
                d = max(kt * P - q0, 0)
                nc.tensor.matmul(
                    out=opsum[:, d:512],
                    lhsT=vp[:, kt, h, :],
                    rhs=pt[:, half * 512 + d:half * 512 + 512],
                    start=(kt == 0),
                    stop=(kt == nkt - 1),
                )

        def emit_normA(qs, h):
            # full-tile approx reciprocal straight from PSUM (custom-DVE ops
            # only work at base partition 0); only row r is meaningful
            opsum = state[(qs, h)]
            r = DH if h % 2 == 0 else 0
            invf = stg.tile([P, 512], F32, name="invf")
            nc.vector.reciprocal_approx_fast(invf[:], opsum[:])
            invb = stg.tile([DH + 1, 512], BF16, name="invb")
            nc.vector.tensor_copy(invb[r:r + 1, :], invf[r:r + 1, :])
            state[(qs, h, "inv")] = (invb, r)

        def emit_normB(qs, h):
            opsum = state.pop((qs, h))
            invb, r = state.pop((qs, h, "inv"))
            jbh, par = h // 2, h % 2
            q0 = qs * 512
            rows = slice(0, DH) if par == 0 else slice(DH, P)
            if par == 1:
                # denom at partition 0: gpsimd broadcast (full-width out; only
                # base-0 in / base-0 out are implemented in the Q7 ucode)
                bsb = stg.tile([P, 512], BF16, name="bsb")
                nc.gpsimd.partition_broadcast(bsb[:, :], invb[0:1, :])
                nc.vector.tensor_mul(oT[rows, jbh, q0:q0 + 512], opsum[rows, :], bsb[rows, :])
            else:
                # denom at partition 64: rank-1 PE broadcast (verified at base 64)
                bcast = prj.tile([P, 512], F32, name="bcast", tag="prj")
                nc.tensor.matmul(out=bcast[:], lhsT=oselB[64:65, 0, :],
                                 rhs=invb[64:65, :], start=True, stop=True)
                bsb = stg.tile([P, 512], BF16, name="bsb")
                nc.vector.tensor_copy(bsb[rows, :], bcast[rows, :])
                nc.vector.tensor_mul(oT[rows, jbh, q0:q0 + 512], opsum[rows, :], bsb[rows, :])

        def emit_ytile(qs, tt, e):
            yt = ysb.tile([P, D], BF16, name="yt")
            for jh in range(2):
                yp = prj.tile([P, 512], F32, name="yp", tag="prj")
                for fc in range(2):
                    nc.tensor.matmul(
                        out=yp[:],
                        lhsT=oT[:, fc, tt * P:(tt + 1) * P],
                        rhs=wo2[:, fc, jh * 512:(jh + 1) * 512],
                        start=(fc == 0),
                        stop=(fc == 1),
                    )
                nc.vector.tensor_copy(yt[:, jh * 512:(jh + 1) * 512], yp[:])
            eng = nc.sync if e % 2 == 0 else nc.scalar
            eng.dma_start(out=y_d[tt * P:(tt + 1) * P, :], in_=yt[:])

        # ---- fused pipeline ----
        steps = []
        for qs in range(QS):
            for h in range(HG):
                nkt = 4 * qs + 4
                for kp in range(nkt // 2):
                    steps.append((qs, h, kp, nkt))
        first_step_of_qs = {}
        for i, (qs, h, kp, nkt) in enumerate(steps):
            if (h, kp) == (0, 0):
                first_step_of_qs[i] = qs

        todo = []

        def flush(i):
            while todo and todo[0][0] <= i:
                todo.pop(0)[1]()

        nsteps = len(steps)
        for i in range(nsteps):
            if i in first_step_of_qs:
                for vj, fn_ in enumerate(emit_slab(first_step_of_qs[i])):
                    todo.append((i + vj, fn_))
                todo.sort(key=lambda e: e[0])
            qs, h, kp, nkt = steps[i]
            emit_S(qs, h, kp)
            flush(i)
            if i >= 1:
                pqs, ph_, pkp, _ = steps[i - 1]
                emit_exp(pqs, ph_, pkp)
            if i >= 2:
                pqs, ph_, pkp, pnkt = steps[i - 2]
                emit_AV(pqs, ph_, pkp, pnkt)
                if pkp == pnkt // 2 - 1:
                    emit_normA(pqs, ph_)
                    todo.append((i + min(5, 2 * (pqs + 1) + 1), lambda q=pqs, hh=ph_: emit_normB(q, hh)))
                    if ph_ == HG - 1:
                        for j, tt in enumerate(range(4 * pqs, 4 * pqs + 4)):
                            todo.append((i + 6 + j,
                                         lambda q=pqs, t_=tt, e=j: emit_ytile(q, t_, e)))
        # drain: last exp/AV, then overlap the final norm chain with the
        # fc0 halves of the last q-slab's output tiles (keeps the PE warm)
        emit_exp(*steps[nsteps - 1][:3])
        last_norm = None
        for i in (nsteps - 2, nsteps - 1):
            qs, h, kp, nkt = steps[i]
            emit_AV(qs, h, kp, nkt)
            if kp == nkt // 2 - 1:
                emit_normA(qs, h)
                last_norm = (qs, h)
        for _, fn in todo:
            fn()
        emit_normB(*last_norm)
        lqs = QS - 1
        for j, tt in enumerate(range(4 * lqs, 4 * lqs + 4)):
            emit_ytile(lqs, tt, j)

    nc.compile()
    return nc


def make_core_inputs(x, Wq, Wk, Wv, Wo):
    import ml_dtypes
    bf = ml_dtypes.bfloat16

    tri = np.triu(np.ones((P, P), dtype=np.float32)).astype(bf)
    # vp init pattern: zeros + ones column (even heads col 64, odd heads col 0)
    vpi = np.zeros((P, TT, HG, P), dtype=np.float32)
    for h in range(HG):
        vpi[:, :, h, DH if h % 2 == 0 else 0] = 1.0
    vpi = vpi.astype(bf)

    def pmajor(w, chunks):  # [chunks*P, f] -> [P, chunks, f]
        return np.ascontiguousarray(
            w.reshape(chunks, P, -1).transpose(1, 0, 2)).astype(bf)

    in_maps = []
    for c in range(NCORES):
        b, hg = c // HG, c % HG
        s = slice(hg * F, (hg + 1) * F)
        # x^T partition-major: [D, T] -> [P, DC, T]
        xt = np.ascontiguousarray(
            x[b].T.reshape(DC, P, T).transpose(1, 0, 2)).astype(bf)
        in_maps.append({
            "xt": xt,
            "wq": pmajor(Wq[:, s], DC),
            "wk": pmajor(Wk[:, s], DC),
            "wv": pmajor(Wv[:, s], DC),
            "wo": pmajor(Wo[s, :], 2),
            "tri": tri,
            "vpi": vpi,
        })
    return in_maps


_NC_CACHE = None


def _get_nc():
    global _NC_CACHE
    if _NC_CACHE is None:
        _NC_CACHE = build_nc()
    return _NC_CACHE


def kernel(x, Wq, Wk, Wv, Wo):
    global LAST_RESULTS
    _install_ntff_hook()
    from concourse.bass_utils import run_bass_kernel_spmd

    x = np.asarray(x, dtype=np.float32)
    Wq = np.asarray(Wq, dtype=np.float32)
    Wk = np.asarray(Wk, dtype=np.float32)
    Wv = np.asarray(Wv, dtype=np.float32)
    Wo = np.asarray(Wo, dtype=np.float32)

    nc = _get_nc()
    in_maps = make_core_inputs(x, Wq, Wk, Wv, Wo)
    res = run_bass_kernel_spmd(nc, in_maps, list(range(NCORES)))
    LAST_RESULTS = res

    out = np.zeros((B, T, D), dtype=np.float32)
    for c in range(NCORES):
        out[c // HG] += np.asarray(res.results[c]["y"], dtype=np.float32)
    return out


# revision 23
# speedup vs baseline: 1.0013x; 1.0013x over previous
"""Causal self-attention (B=2, T=2048, D=1024, H=16) on 8 TRN2 NeuronCores.

Sharding: data-parallel over batch (2) x tensor-parallel over head groups (4).
Each core handles 1 batch x 4 heads: Wq/Wk/Wv column-sharded, Wo row-sharded;
each core emits a partial (T, D) output and the host sums 4 partials per batch.

v2 design (vs the fp32r baseline):
  - x is transposed on the HOST and shipped as bf16 [P, DC, T] partition-major,
    eliminating all 128 PE transposes and their PSUM evictions.
  - All matmul operands are bf16 (fp32 PSUM accumulation): enables the PE's
    fast-weight-load path (fp32r blocks FWL), halves eviction/DMA bytes.
  - No mask-inject matmuls: diagonal S tiles compute only columns [d, 512);
    exp is restricted to the written PSUM region (split calls on diagonal
    k-pairs), and the in-tile causal triangle is applied post-exp as a bf16
    tensor_mul with a host-provided [128,128] upper-triangular mask.
  - All PSUM evictions run on DVE; the scalar engine does (almost) only exp.
  - Softmax denominators ride in V' ones-columns (parity layout: even heads
    col 64 / rows 0-63 data, odd heads col 0 / rows 64-127 data); per (qs,h)
    the denom row is folded via SBUF DMA, reciprocal'd on DVE, unfolded and
    rank-1-broadcast on PE, then fused into the oT eviction multiply.
"""

import sys, os, types

sys.path.insert(0, "/opt/trn_rl_repo")

import numpy as np
from contextlib import ExitStack

import concourse.bass as bass
import concourse.mybir as mybir
import concourse.tile as tile
from concourse import bacc

B, T, D, H = 2, 2048, 1024, 16
DH = D // H          # 64
NCORES = 8
HG = 4               # heads per core
F = HG * DH          # 256 local features per core
P = 128
F32 = mybir.dt.float32
F32R = mybir.dt.float32r
BF16 = mybir.dt.bfloat16
FP8 = mybir.dt.float8e4

TT = T // P          # 16 t-tiles
QS = T // 512        # 4 q-slabs
DC = D // P          # 8 d-chunks

LAST_RESULTS = None  # BassKernelResults of the most recent hardware run


def _install_ntff_hook():
    if "antenv.axon_hooks" in sys.modules:
        return
    try:
        import antenv
        from trn_agent_boot.trn_boot import _ntff_profile_via_ctypes

        m = types.ModuleType("antenv.axon_hooks")
        h = _ntff_profile_via_ctypes("/opt/axon/libaxon_pjrt.so")
        m.get_axon_ntff_profile_hook = lambda: h
        m.set_axon_ntff_profile_hook = lambda hh: None
        sys.modules["antenv.axon_hooks"] = m
        antenv.axon_hooks = m
    except Exception:
        pass


def build_nc():
    nc = bacc.Bacc("TRN2", target_bir_lowering=False, debug=False)

    xt_d = nc.dram_tensor("xt", [P, DC, T], BF16, kind="ExternalInput").ap()
    wq_d = nc.dram_tensor("wq", [P, DC, F], BF16, kind="ExternalInput").ap()
    wk_d = nc.dram_tensor("wk", [P, DC, F], BF16, kind="ExternalInput").ap()
    wv_d = nc.dram_tensor("wv", [P, DC, F], BF16, kind="ExternalInput").ap()
    wo_d = nc.dram_tensor("wo", [P, 2, D], BF16, kind="ExternalInput").ap()
    tri_d = nc.dram_tensor("tri", [P, P], BF16, kind="ExternalInput").ap()
    vpi_d = nc.dram_tensor("vpi", [P, TT, HG, P], BF16, kind="ExternalInput").ap()
    y_d = nc.dram_tensor("y", [T, D], BF16, kind="ExternalOutput").ap()

    with tile.TileContext(nc) as tc, ExitStack() as ctx:
        const = ctx.enter_context(tc.tile_pool(name="const", bufs=1))
        wpool = ctx.enter_context(tc.tile_pool(name="wpool", bufs=1))
        qkv = ctx.enter_context(tc.tile_pool(name="qkv", bufs=1))
        xsl = ctx.enter_context(tc.tile_pool(name="xsl", bufs=2))
        sp_ps = ctx.enter_context(tc.tile_pool(name="sp_ps", bufs=3, space="PSUM"))
        o_ps = ctx.enter_context(tc.tile_pool(name="o_ps", bufs=2, space="PSUM"))
        ptp = ctx.enter_context(tc.tile_pool(name="ptp", bufs=4))
        stg = ctx.enter_context(tc.tile_pool(name="stg", bufs=6))
        ysb = ctx.enter_context(tc.tile_pool(name="ysb", bufs=4))

        # ---- constants / warmups ----
        tri = const.tile([P, P], BF16, name="tri")
        # dummy matmul burst: ~4.5us of PE activity during the DMA-bound
        # preamble flips the HAM clock gate to 8/8 before real work arrives
        wsrc = const.tile([P, P], BF16, name="wsrc")
        nc.vector.memset(wsrc[:], 0.0)
        wps = sp_ps.tile([P, 512], F32, name="wps", tag="sp")
        for _ in range(40):
            nc.tensor.matmul(out=wps[:, 0:P], lhsT=wsrc[:], rhs=wsrc[:],
                             start=True, stop=True)
        # touch Exp early so the ACT table load happens in the idle preamble
        warm_src = const.tile([1, 1], F32, name="warm_src")
        nc.vector.memset(warm_src[:], 0.0)
        warm = const.tile([1, 1], F32, name="warm")
        nc.scalar.activation(warm[:], warm_src[:], mybir.ActivationFunctionType.Exp)
        # touch partition_broadcast early so the gpsimd library IRAM load
        # (~6us) happens in the idle preamble
        wpb = const.tile([P, 64], BF16, name="wpb")
        nc.vector.memset(wpb[0:1, :], 1.0)
        nc.gpsimd.partition_broadcast(wpb[:, :], wpb[0:1, :])
        # denominator-broadcast selectors for the K=1 PE path (base-64 rows):
        # bcast = oselB[r]^T (x) inv[r]; even heads r=64 -> out rows 0:64
        oselB = const.tile([65, 2, P], BF16, name="oselB")
        nc.vector.memset(oselB[0:1, :, :], 0.0)
        nc.vector.memset(oselB[64:65, :, :], 0.0)
        nc.vector.memset(oselB[64:65, 0, 0:64], 1.0)

        # ---- persistent tensors ----
        wq_s = wpool.tile([P, DC, F], BF16, name="wq_s")
        wk_s = wpool.tile([P, DC, F], BF16, name="wk_s")
        wv_s = wpool.tile([P, DC, F], BF16, name="wv_s")
        wo2 = wpool.tile([P, 2, D], BF16, name="wo2")
        qT = qkv.tile([P, 2, T], BF16, name="qT")        # [2 heads x dh, jb, t]
        kTz0 = qkv.tile([P, 2, T], BF16, name="kTz0")    # [k_even; 0]
        kTz1 = qkv.tile([P, 2, T], BF16, name="kTz1")    # [0; k_odd]
        vp = qkv.tile([P, TT, HG, P], BF16, name="vp")   # padded V', parity layouts
        oT = qkv.tile([P, 2, T], BF16, name="oT")        # normalized o^T [f, t]

        # ---- preamble DMAs + fills ----
        # Only what gates the first projections ships immediately: x slab 0
        # (sync ring), wq/wk/tri (scalar ring), vp init for q-slab 0's
        # k-tiles (gpsimd ring).  Everything else (x slab 1, wv, wo, rest of
        # vp init) is sequenced behind slab 0's arrival with tiny
        # dependency-creating writes, so the SDMA engines don't split early
        # bandwidth 6 ways and delay the pipeline start.
        xs_next = xsl.tile([P, DC, 512], BF16, name="xs")
        for c2 in range(4):
            nc.sync.dma_start(out=xs_next[:, 2 * c2:2 * c2 + 2, :],
                              in_=xt_d[:, 2 * c2:2 * c2 + 2, 0:512])
        nc.scalar.dma_start(out=wq_s[:, 0:4], in_=wq_d[:, 0:4])
        nc.scalar.dma_start(out=wq_s[:, 4:8], in_=wq_d[:, 4:8])
        nc.scalar.dma_start(out=wk_s[:, 0:4], in_=wk_d[:, 0:4])
        nc.scalar.dma_start(out=wk_s[:, 4:8], in_=wk_d[:, 4:8])
        nc.scalar.dma_start(out=tri[:], in_=tri_d[:])
        nc.vector.memset(kTz0[64:128, :, :], 0.0)
        nc.vector.memset(kTz1[0:64, :, :], 0.0)
        xs0 = xs_next
        for tgt in (vp[0:1, 0, 0, 0:1], vp[0:1, 4, 0, 0:1],
                    wv_s[0:1, 0, 0:1], wo2[0:1, 0, 0:1]):
            nc.vector.tensor_copy(tgt, xs0[0:1, 0, 0:1])
        nc.gpsimd.dma_start(out=vp[:, 0:4], in_=vpi_d[:, 0:4])
        nc.gpsimd.dma_start(out=vp[:, 4:TT], in_=vpi_d[:, 4:TT])
        nc.scalar.dma_start(out=wv_s[:], in_=wv_d[:])
        nc.scalar.dma_start(out=wo2[:], in_=wo_d[:])

        # ---- emission helpers ----
        def emit_slab(ts):
            nonlocal xs_next
            xs = xs_next
            if ts + 1 < QS:
                xs_next = xsl.tile([P, DC, 512], BF16, name="xs")
                if ts == 0:
                    # keep slab 1's transfer out of the startup DMA window
                    nc.vector.tensor_copy(xs_next[0:1, 0, 0:1], xs[0:1, 0, 0:1])
                nc.sync.dma_start(out=xs_next[:], in_=xt_d[:, :, (ts + 1) * 512:(ts + 2) * 512])
            sl = slice(ts * 512, (ts + 1) * 512)

            def emit_proj(w_s, which, jb):
                pp = sp_ps.tile([P, 512], F32, name="pp", tag="sp")
                for dc in range(DC):
                    nc.tensor.matmul(
                        out=pp[:],
                        lhsT=w_s[:, dc, jb * P:(jb + 1) * P],
                        rhs=xs[:, dc, :],
                        start=(dc == 0),
                        stop=(dc == DC - 1),
                    )
                if which == "q":
                    nc.vector.tensor_copy(qT[:, jb, sl], pp[:])
                else:
                    nc.vector.tensor_copy(kTz0[0:64, jb, sl], pp[0:64, :])
                    nc.vector.tensor_copy(kTz1[64:128, jb, sl], pp[64:128, :])

            for jb in range(2):
                emit_proj(wq_s, "q", jb)
            deferred = []
            if ts == 0:
                for jb in range(2):
                    emit_proj(wk_s, "k", jb)
            else:
                for jb in range(2):
                    deferred.append(lambda b=jb: emit_proj(wk_s, "k", b))

            def emit_v(j, tt):
                pv = sp_ps.tile([P, F], F32, name="pv", tag="sp")
                for dc in range(DC):
                    nc.tensor.matmul(
                        out=pv[:],
                        lhsT=xs[:, dc, j * P:(j + 1) * P],
                        rhs=wv_s[:, dc, :],
                        start=(dc == 0),
                        stop=(dc == DC - 1),
                    )
                pvv = pv[:].rearrange("p (hp par dh) -> p hp par dh", hp=2, par=2, dh=DH)
                ve = vp[:, tt, :, :].rearrange("p (hp par) c -> p hp par c", par=2)
                nc.vector.tensor_copy(ve[:, :, 0, 0:DH], pvv[:, :, 0, :])
                nc.vector.tensor_copy(ve[:, :, 1, DH:P], pvv[:, :, 1, :])
            return deferred + [(lambda a=j_, b=tt_: emit_v(a, b)) for j_, tt_ in enumerate(range(4 * ts, 4 * ts + 4))]

        state = {}

        def emit_S(qs, h, kp):
            jbh, par = h // 2, h % 2
            kTz = kTz0 if par == 0 else kTz1
            q0 = qs * 512
            spair = sp_ps.tile([P, 1024], F32, name="spair", tag="sp")
            for half in range(2):
                kt = 2 * kp + half
                k0 = kt * P
                sreg = spair[:, half * 512:(half + 1) * 512]
                lhsk = kTz[:, jbh, k0:k0 + P]
                rhsq = qT[:, jbh, :]
                d = k0 - q0
                if d > 0:
                    nc.tensor.matmul(out=sreg[:, d:512], lhsT=lhsk,
                                     rhs=rhsq[:, q0 + d:q0 + 512],
                                     start=True, stop=True)
                else:
                    nc.tensor.matmul(out=sreg, lhsT=lhsk,
                                     rhs=rhsq[:, q0:q0 + 512],
                                     start=True, stop=True)
            state[(qs, h, kp)] = spair

        def emit_exp(qs, h, kp):
            spair = state[(qs, h, kp)]
            q0 = qs * 512
            pt = ptp.tile([P, 1024], BF16, name="pt")
            d1 = (2 * kp + 1) * P - q0
            if d1 > 0:
                # diagonal pair: exp only the written PSUM regions
                d0 = max(d1 - P, 0)
                nc.scalar.activation(pt[:, d0:512], spair[:, d0:512],
                                     mybir.ActivationFunctionType.Exp, scale=0.125)
                nc.scalar.activation(pt[:, 512 + d1:1024], spair[:, 512 + d1:1024],
                                     mybir.ActivationFunctionType.Exp, scale=0.125)
            else:
                nc.scalar.activation(pt[:], spair[:],
                                     mybir.ActivationFunctionType.Exp, scale=0.125)
            # in-tile causal triangle on diagonal k-tiles
            for half in range(2):
                d = (2 * kp + half) * P - q0
                if d >= 0:
                    c = half * 512 + d
                    nc.vector.tensor_mul(pt[:, c:c + P], pt[:, c:c + P], tri[:])
            state[(qs, h, kp)] = (spair, pt)

        def emit_AV(qs, h, kp, nkt):
            _, pt = state.pop((qs, h, kp))
            q0 = qs * 512
            if kp == 0:
                state[(qs, h)] = o_ps.tile([P, 512], F32, name="opsum")
            opsum = state[(qs, h)]
            for half in range(2):
                kt = 2 * kp + half
                d = max(kt * P - q0, 0)
                nc.tensor.matmul(
                    out=opsum[:, d:512],
                    lhsT=vp[:, kt, h, :],
                    rhs=pt[:, half * 512 + d:half * 512 + 512],
                    start=(kt == 0),
                    stop=(kt == nkt - 1),
                )

        def emit_normA(qs, h):
            # full-tile approx reciprocal straight from PSUM (custom-DVE ops
            # only work at base partition 0); only row r is meaningful
            opsum = state[(qs, h)]
            r = DH if h % 2 == 0 else 0
            invf = stg.tile([P, 512], F32, name="invf")
            nc.vector.reciprocal_approx_fast(invf[:], opsum[:])
            invb = stg.tile([DH + 1, 512], BF16, name="invb")
            nc.vector.tensor_copy(invb[r:r + 1, :], invf[r:r + 1, :])
            state[(qs, h, "inv")] = (invb, r)

        def emit_normB(qs, h):
            opsum = state.pop((qs, h))
            invb, r = state.pop((qs, h, "inv"))
            jbh, par = h // 2, h % 2
            q0 = qs * 512
            rows = slice(0, DH) if par == 0 else slice(DH, P)
            if par == 1:
                # denom at partition 0: gpsimd broadcast (full-width out; only
                # base-0 in / base-0 out are implemented in the Q7 ucode)
                bsb = stg.tile([P, 512], BF16, name="bsb")
                nc.gpsimd.partition_broadcast(bsb[:, :], invb[0:1, :])
                nc.vector.tensor_mul(oT[rows, jbh, q0:q0 + 512], opsum[rows, :], bsb[rows, :])
            else:
                # denom at partition 64: rank-1 PE broadcast (verified at base 64)
                bcast = sp_ps.tile([P, 512], F32, name="bcast", tag="sp")
                nc.tensor.matmul(out=bcast[:], lhsT=oselB[64:65, 0, :],
                                 rhs=invb[64:65, :], start=True, stop=True)
                bsb = stg.tile([P, 512], BF16, name="bsb")
                nc.vector.tensor_copy(bsb[rows, :], bcast[rows, :])
                nc.vector.tensor_mul(oT[rows, jbh, q0:q0 + 512], opsum[rows, :], bsb[rows, :])

        def emit_ytile(qs, tt, e, drain=False):
            yp = sp_ps.tile([P, 1024], F32, name="yp", tag="sp")
            for jh in range(2):
                for fc in range(2):
                    nc.tensor.matmul(
                        out=yp[:, jh * 512:(jh + 1) * 512],
                        lhsT=oT[:, fc, tt * P:(tt + 1) * P],
                        rhs=wo2[:, fc, jh * 512:(jh + 1) * 512],
                        start=(fc == 0),
                        stop=(fc == 1),
                    )
            yt = ysb.tile([P, D], BF16, name="yt")
            nc.vector.tensor_copy(yt[:], yp[:])
            eng = nc.scalar if (drain and e % 2 == 1) else nc.sync
            eng.dma_start(out=y_d[tt * P:(tt + 1) * P, :], in_=yt[:])

        # ---- fused pipeline ----
        steps = []
        for qs in range(QS):
            for h in range(HG):
                nkt = 4 * qs + 4
                for kp in range(nkt // 2):
                    steps.append((qs, h, kp, nkt))
        first_step_of_qs = {}
        for i, (qs, h, kp, nkt) in enumerate(steps):
            if (h, kp) == (0, 0):
                first_step_of_qs[i] = qs

        todo = []

        def flush(i):
            while todo and todo[0][0] <= i:
                todo.pop(0)[1]()

        nsteps = len(steps)
        for i in range(nsteps):
            if i in first_step_of_qs:
                for vj, fn_ in enumerate(emit_slab(first_step_of_qs[i])):
                    todo.append((i + vj, fn_))
                todo.sort(key=lambda e: e[0])
            qs, h, kp, nkt = steps[i]
            emit_S(qs, h, kp)
            flush(i)
            if i >= 1:
                pqs, ph_, pkp, _ = steps[i - 1]
                emit_exp(pqs, ph_, pkp)
            if i >= 2:
                pqs, ph_, pkp, pnkt = steps[i - 2]
                emit_AV(pqs, ph_, pkp, pnkt)
                if pkp == pnkt // 2 - 1:
                    emit_normA(pqs, ph_)
                    todo.append((i + min(5, 2 * (pqs + 1) + 1), lambda q=pqs, hh=ph_: emit_normB(q, hh)))
                    if ph_ == HG - 1:
                        for j, tt in enumerate(range(4 * pqs, 4 * pqs + 4)):
                            todo.append((i + 6 + j,
                                         lambda q=pqs, t_=tt, e=j: emit_ytile(q, t_, e)))
        # drain: last exp/AV, then overlap the final norm chain with the
        # fc0 halves of the last q-slab's output tiles (keeps the PE warm)
        emit_exp(*steps[nsteps - 1][:3])
        last_norm = None
        for i in (nsteps - 2, nsteps - 1):
            qs, h, kp, nkt = steps[i]
            emit_AV(qs, h, kp, nkt)
            if kp == nkt // 2 - 1:
                emit_normA(qs, h)
                last_norm = (qs, h)
        for _, fn in todo:
            fn()
        emit_normB(*last_norm)
        lqs = QS - 1
        for j, tt in enumerate(range(4 * lqs, 4 * lqs + 4)):
            emit_ytile(lqs, tt, j, drain=True)

    nc.compile()
    return nc


def make_core_inputs(x, Wq, Wk, Wv, Wo):
    import ml_dtypes
    bf = ml_dtypes.bfloat16

    tri = np.triu(np.ones((P, P), dtype=np.float32)).astype(bf)
    # vp init pattern: zeros + ones column (even heads col 64, odd heads col 0)
    vpi = np.zeros((P, TT, HG, P), dtype=np.float32)
    for h in range(HG):
        vpi[:, :, h, DH if h % 2 == 0 else 0] = 1.0
    vpi = vpi.astype(bf)

    def pmajor(w, chunks):  # [chunks*P, f] -> [P, chunks, f]
        return np.ascontiguousarray(
            w.reshape(chunks, P, -1).transpose(1, 0, 2)).astype(bf)

    in_maps = []
    for c in range(NCORES):
        b, hg = c // HG, c % HG
        s = slice(hg * F, (hg + 1) * F)
        # x^T partition-major: [D, T] -> [P, DC, T]
        xt = np.ascontiguousarray(
            x[b].T.reshape(DC, P, T).transpose(1, 0, 2)).astype(bf)
        in_maps.append({
            "xt": xt,
            "wq": pmajor(Wq[:, s], DC),
            "wk": pmajor(Wk[:, s], DC),
            "wv": pmajor(Wv[:, s], DC),
            "wo": pmajor(Wo[s, :], 2),
            "tri": tri,
            "vpi": vpi,
        })
    return in_maps


_NC_CACHE = None


def _get_nc():
    global _NC_CACHE
    if _NC_CACHE is None:
        _NC_CACHE = build_nc()
    return _NC_CACHE


def kernel(x, Wq, Wk, Wv, Wo):
    global LAST_RESULTS
    _install_ntff_hook()
    from concourse.bass_utils import run_bass_kernel_spmd

    x = np.asarray(x, dtype=np.float32)
    Wq = np.asarray(Wq, dtype=np.float32)
    Wk = np.asarray(Wk, dtype=np.float32)
    Wv = np.asarray(Wv, dtype=np.float32)
    Wo = np.asarray(Wo, dtype=np.float32)

    nc = _get_nc()
    in_maps = make_core_inputs(x, Wq, Wk, Wv, Wo)
    res = run_bass_kernel_spmd(nc, in_maps, list(range(NCORES)))
    LAST_RESULTS = res

    out = np.zeros((B, T, D), dtype=np.float32)
    for c in range(NCORES):
        out[c // HG] += np.asarray(res.results[c]["y"], dtype=np.float32)
    return out
